# revision 17
# baseline (speedup 1.0000x reference)
"""Trainium2 Bass kernel for nn_CrossAttention (B=4, Lq=Lk=2048, D=1024, H=16, d=64).

Sharding: 8 cores = 4 batches x 2 head-groups (8 heads each).
Each core computes a partial out^T = Wout_g^T @ y_g^T for its (batch, head-group);
host sums the two head-group partials per batch and transposes.

All matmul operands are bf16 (fp32 PSUM accumulation). Device layout is
feature-major ("T" = [feature, seq]):
  qT/kT: [512, L] bf16 (8 heads x 64 dims on partitions, seq on free axis)
  S^T:   [k, q] tiles -> softmax sum via an appended ones-column in v (M=65)
  exp:   ACT, with the k-side RMSNorm rstd (and the 1/sqrt(d) scale) folded
         into the per-partition activation scale operand.
RoPE rotate-half comes from a second projection against host-permuted weights
(Wq_rot/Wk_rot); the C/S combine runs on the vector engine straight out of
PSUM. The q projection runs fb-outer so qT[0] completes early and attention
matmuls chase the projection without a pipeline bubble.
"""
import os
import numpy as np
from contextlib import ExitStack

import ml_dtypes

import concourse.bass as bass
import concourse.tile as tile
from concourse import bacc, mybir
from concourse.bass_utils import run_bass_kernel_spmd

F32 = mybir.dt.float32
BF16 = mybir.dt.bfloat16
EXP = mybir.ActivationFunctionType.Exp
SQUARE = mybir.ActivationFunctionType.Square
SQRT = mybir.ActivationFunctionType.Sqrt
COPY = mybir.ActivationFunctionType.Copy

D = 1024          # model dim
L = 2048          # seq len (q and k)
HC = 8            # heads per core
DH = 64           # head dim
F = HC * DH       # 512 local features
N_CORES = 8
EPS = float(np.finfo(np.float32).eps)

LAST_RESULTS = None  # BassKernelResults of the most recent run (for test harness)
_NC = None


# --------------------------------------------------------------------------- #
# Device program
# --------------------------------------------------------------------------- #

def _norm_rope_chunk(tc, pool, pps, dst, ps, rot, c_sb, s_sb, bdiag, bmap,
                     eps_t, side, fb, col0, rk_dram):
    """RMSNorm + RoPE for one projected [128, 512] chunk, out of PSUM."""
    nc = tc.nc
    # sum of squares over each head's 64 partition rows
    sq = pool.tile([128, 512], BF16, tag="sq", bufs=2)
    nc.scalar.activation(sq[:], ps[:], SQUARE)
    vps = pps.tile([2, 512], F32, tag="var", bufs=2)
    nc.tensor.matmul(vps[:], bdiag[:], sq[:], start=True, stop=True)
    std = pool.tile([2, 512], F32, tag="std", bufs=2)
    rstd = pool.tile([2, 512], F32, tag="rstd", bufs=2)
    if side == "q":
        # std = sqrt(raw/64 + eps); rstd = 1/std
        nc.scalar.activation(std[:], vps[:], SQRT,
                             bias=eps_t[:], scale=1.0 / 64.0)
        nc.vector.reciprocal_approx_fast(out=rstd[:], in_=std[:])
        rstd_r = pool.tile([2, 512], BF16, tag="rstdr", bufs=2)
        nc.vector.tensor_copy(rstd_r[:], rstd[:])
        bps = pps.tile([128, 512], F32, tag="bc", bufs=2)
        nc.tensor.matmul(bps[:], bmap[:], rstd_r[:], start=True, stop=True)
    else:
        # fold the 1/8 attention scale: rk = 1/(8*std) = 1/sqrt(64*(raw/64+eps))
        nc.scalar.activation(std[:], vps[:], SQRT, bias=eps_t[:], scale=1.0)
        nc.vector.reciprocal_approx_fast(out=rstd[:], in_=std[:])
        nc.sync.dma_start(
            rk_dram[2 * fb:2 * fb + 2, col0:col0 + 512], rstd[:])
    # rope combine on the vector engine, reading PSUM directly
    t2 = pool.tile([128, 512], BF16, tag="t2", bufs=2)
    nc.vector.tensor_mul(t2[:], ps[:], c_sb[:, col0:col0 + 512])
    tmp = pool.tile([128, 512], BF16, tag="tmp", bufs=2)
    nc.vector.tensor_mul(tmp[:], rot[:], s_sb[:, col0:col0 + 512])
    chunk = dst[fb][:, col0:col0 + 512]
    if side == "q":
        t3 = pool.tile([128, 512], BF16, tag="t3", bufs=2)
        nc.vector.tensor_add(t3[:], t2[:], tmp[:])
        nc.vector.tensor_mul(chunk, t3[:], bps[:])
    else:
        nc.vector.tensor_add(chunk, t2[:], tmp[:])


def _proj_pair(nc, pps, w_sb, wr_sb, x_ap_chunks):
    """Dual projection (plain + rotated weights) of one 512-col chunk."""
    ps = pps.tile([128, 512], F32, tag="proj", bufs=2)
    for dc in range(8):
        nc.tensor.matmul(ps[:], w_sb[dc], x_ap_chunks[dc],
                         start=(dc == 0), stop=(dc == 7))
    rot = pps.tile([128, 512], F32, tag="rot", bufs=2)
    for dc in range(8):
        nc.tensor.matmul(rot[:], wr_sb[dc], x_ap_chunks[dc],
                         start=(dc == 0), stop=(dc == 7))
    return ps, rot


def _build_program():
    nc = bacc.Bacc("TRN2", target_bir_lowering=False, debug=False,
                   num_devices=N_CORES)
    dt = nc.dram_tensor
    xqT = dt("xqT", (D, L), BF16, kind="ExternalInput").ap()
    xkvT = dt("xkvT", (D, L), BF16, kind="ExternalInput").ap()
    wq = dt("wq", (D, F), BF16, kind="ExternalInput").ap()
    wqr = dt("wqr", (D, F), BF16, kind="ExternalInput").ap()
    wk = dt("wk", (D, F), BF16, kind="ExternalInput").ap()
    wkr = dt("wkr", (D, F), BF16, kind="ExternalInput").ap()
    wv = dt("wv", (D, F), BF16, kind="ExternalInput").ap()
    wout = dt("wout", (F, D), BF16, kind="ExternalInput").ap()
    cq = dt("cq", (128, L), BF16, kind="ExternalInput").ap()
    sq_t = dt("sq", (128, L), BF16, kind="ExternalInput").ap()
    ck = dt("ck", (128, L), BF16, kind="ExternalInput").ap()
    sk_t = dt("sk", (128, L), BF16, kind="ExternalInput").ap()
    bdiag_d = dt("bdiag", (128, 2), BF16, kind="ExternalInput").ap()
    bmap_d = dt("bmap", (2, 128), BF16, kind="ExternalInput").ap()
    selA_d = dt("selA", (128, 128), BF16, kind="ExternalInput").ap()
    selB_d = dt("selB", (128, 128), BF16, kind="ExternalInput").ap()
    outT = dt("outT", (D, L), F32, kind="ExternalOutput").ap()

    with tile.TileContext(nc) as tc:
        with ExitStack() as ctx:
            big = ctx.enter_context(tc.tile_pool(name="big", bufs=1))
            dram = ctx.enter_context(tc.tile_pool(name="dram", bufs=1, space="DRAM"))

            kT = [big.tile([128, L], BF16, tag=f"kT{i}", name=f"kT{i}") for i in range(4)]
            qT = [big.tile([128, L], BF16, tag=f"qT{i}", name=f"qT{i}") for i in range(4)]
            vaug = [big.tile([128, HC * 65], BF16, tag=f"v{i}", name=f"vaug{i}") for i in range(16)]
            rk_dram = dram.tile([HC, L], F32, tag="rk")

            bdiag = big.tile([128, 2], BF16, tag="bdiag")
            nc.sync.dma_start(bdiag[:], bdiag_d[:])
            bmap = big.tile([2, 128], BF16, tag="bmap")
            nc.sync.dma_start(bmap[:], bmap_d[:])

            # first DMAs in the queue: the xkv half phase A starts on
            xpool = ctx.enter_context(tc.tile_pool(name="xp", bufs=1))
            xkv_h0 = []
            for dc in range(8):
                x = xpool.tile([128, 1024], BF16, tag=f"xkv{dc}", bufs=1)
                nc.sync.dma_start(x[:], xkvT[dc * 128:(dc + 1) * 128, 0:1024])
                xkv_h0.append(x)

            # weights [128, F] x8 per projection; phase-A inputs load first
            def load_w(w_dram, name):
                tiles = []
                for dc in range(8):
                    w = big.tile([128, F], BF16, tag=f"{name}{dc}",
                                 name=f"{name}{dc}")
                    nc.sync.dma_start(w[:], w_dram[dc * 128:(dc + 1) * 128, :])
                    tiles.append(w)
                return tiles
            wk_sb = load_w(wk, "wk")
            wkr_sb = load_w(wkr, "wkr")
            wv_sb = load_w(wv, "wv")
            ck_sb = big.tile([128, L], BF16, tag="ckt")
            nc.sync.dma_start(ck_sb[:], ck[:])
            sk_sb = big.tile([128, L], BF16, tag="skt")
            nc.sync.dma_start(sk_sb[:], sk_t[:])
            # phase-B inputs queue behind phase A's (prefetch during A)
            wq_sb = load_w(wq, "wq")
            wqr_sb = load_w(wqr, "wqr")
            cq_sb = big.tile([128, L], BF16, tag="cqt")
            nc.sync.dma_start(cq_sb[:], cq[:])
            sq_sb = big.tile([128, L], BF16, tag="sqt")
            nc.sync.dma_start(sq_sb[:], sq_t[:])
            xq_sb = []
            for dc in range(8):
                x = big.tile([128, L], BF16, tag=f"xq{dc}", name=f"xq{dc}")
                nc.sync.dma_start(x[:], xqT[dc * 128:(dc + 1) * 128, :])
                xq_sb.append(x)

            # ---- Phase A: k/v projections (xkv by seq halves) ----
            with ExitStack() as actx:
                apool = actx.enter_context(tc.tile_pool(name="a_sb", bufs=1))
                aps = actx.enter_context(tc.tile_pool(name="a_ps", bufs=1,
                                                      space="PSUM"))
                eps_k = apool.tile([2, 1], F32, tag="epsk")
                nc.gpsimd.memset(eps_k[:], 64.0 * EPS)
                for lh in range(2):
                    if lh == 0:
                        xkv_h = xkv_h0
                    else:
                        xkv_h = []
                        for dc in range(8):
                            x = xpool.tile([128, 1024], BF16, tag=f"xkv{dc}",
                                           bufs=1)
                            nc.sync.dma_start(
                                x[:], xkvT[dc * 128:(dc + 1) * 128,
                                           1024:2048])
                            xkv_h.append(x)
                    for fb in range(4):
                        for qn in range(2):
                            col0 = lh * 1024 + qn * 512
                            ps, rot = _proj_pair(
                                nc, aps,
                                [wk_sb[dc][:, fb * 128:(fb + 1) * 128]
                                 for dc in range(8)],
                                [wkr_sb[dc][:, fb * 128:(fb + 1) * 128]
                                 for dc in range(8)],
                                [xkv_h[dc][:, qn * 512:(qn + 1) * 512]
                                 for dc in range(8)])
                            _norm_rope_chunk(tc, apool, aps, kT, ps, rot,
                                             ck_sb, sk_sb, bdiag, bmap,
                                             eps_k, "k", fb, col0, rk_dram)
                    for lc in range(8):
                        kc = lh * 8 + lc
                        ps = aps.tile([128, 512], F32, tag="proj", bufs=2)
                        for dc in range(8):
                            nc.tensor.matmul(
                                ps[:],
                                xkv_h[dc][:, lc * 128:(lc + 1) * 128],
                                wv_sb[dc][:],
                                start=(dc == 0), stop=(dc == 7))
                        va = vaug[kc]
                        nc.gpsimd.memset(va[:], 1.0)
                        va3 = va.rearrange("p (h c) -> p h c", c=65)
                        ps3 = ps.rearrange("p (h c) -> p h c", c=64)
                        nc.vector.tensor_copy(va3[:, :, 0:64], ps3[:])

            # ---- Phase B: q projection, fb-outer so qT[0] finishes first ----
            with ExitStack() as bctx:
                bpool = bctx.enter_context(tc.tile_pool(name="b_sb", bufs=1))
                bps_p = bctx.enter_context(tc.tile_pool(name="b_ps", bufs=1,
                                                        space="PSUM"))
                eps_q = bpool.tile([2, 1], F32, tag="epsq")
                nc.gpsimd.memset(eps_q[:], EPS)
                for fb in range(4):
                    for cn in range(4):   # 512-col chunks across full L
                        col0 = cn * 512
                        ps, rot = _proj_pair(
                            nc, bps_p,
                            [wq_sb[dc][:, fb * 128:(fb + 1) * 128]
                             for dc in range(8)],
                            [wqr_sb[dc][:, fb * 128:(fb + 1) * 128]
                             for dc in range(8)],
                            [xq_sb[dc][:, col0:col0 + 512]
                             for dc in range(8)])
                        _norm_rope_chunk(tc, bpool, bps_p, qT, ps, rot,
                                         cq_sb, sq_sb, bdiag, bmap,
                                         eps_q, "q", fb, col0, None)

            # ---- Phases C+D persistents ----
            p2 = ctx.enter_context(tc.tile_pool(name="p2", bufs=1))
            ytr = [p2.tile([128, L], BF16, tag=f"ytr{i}", name=f"ytr{i}")
                   for i in range(4)]
            sums_g = [p2.tile([128, L], F32, tag=f"sums{g}", name=f"sums{g}")
                      for g in range(2)]
            nc.gpsimd.memset(sums_g[0][:], 1.0)
            nc.gpsimd.memset(sums_g[1][:], 1.0)
            rs_g = [p2.tile([128, L], BF16, tag=f"rs{g}", name=f"rs{g}")
                    for g in range(2)]

            # ---- Phase C: attention (2 q-passes per head) ----
            with ExitStack() as cctx:
                cpool = cctx.enter_context(tc.tile_pool(name="att_sb", bufs=1))
                cps = cctx.enter_context(
                    tc.tile_pool(name="att_ps", bufs=1, space="PSUM"))
                rk_sb = cpool.tile([128, HC, 16], F32, tag="rk")
                nc.sync.dma_start(
                    rk_sb[:], rk_dram.rearrange("h (kc p) -> p h kc", p=128))
                sel_c = []
                for i, sd in enumerate((selA_d, selB_d)):
                    s = cpool.tile([128, 128], BF16, tag=f"sel{i}",
                                   name=f"sel{i}")
                    nc.sync.dma_start(s[:], sd[:])
                    sel_c.append(s)
                for h in range(HC):
                    fb, off = h // 2, (h % 2) * 64
                    for pss in range(2):
                        yps = [cps.tile([65, 512], F32, tag=f"y{j}", bufs=1,
                                        name=f"yps{h}_{pss}_{j}")
                               for j in range(2)]
                        for kc in range(16):
                            va3 = vaug[kc].rearrange("p (h c) -> p h c", c=65)
                            rk_ap = rk_sb[:, h, kc:kc + 1]
                            sps = cps.tile([128, 1024], F32, tag="s", bufs=2)
                            for j in range(2):
                                qn = pss * 2 + j
                                nc.tensor.matmul(
                                    sps[:, j * 512:(j + 1) * 512],
                                    kT[fb][off:off + 64,
                                           kc * 128:(kc + 1) * 128],
                                    qT[fb][off:off + 64,
                                           qn * 512:(qn + 1) * 512],
                                    start=True, stop=True)
                            pt = cpool.tile([128, 1024], BF16, tag="p", bufs=3)
                            nc.scalar.activation(pt[:], sps[:], EXP,
                                                 scale=rk_ap)
                            for j in range(2):
                                nc.tensor.matmul(
                                    yps[j][:], va3[:, h, :],
                                    pt[:, j * 512:(j + 1) * 512],
                                    start=(kc == 0), stop=(kc == 15))
                        for j in range(2):
                            qn = pss * 2 + j
                            nc.vector.tensor_copy(
                                ytr[fb][off:off + 64,
                                        qn * 512:(qn + 1) * 512],
                                yps[j][0:64, :])
                            slot = 32 * (h % 4)
                            nc.vector.tensor_copy(
                                sums_g[h // 4][slot:slot + 1,
                                               qn * 512:(qn + 1) * 512],
                                yps[j][64:65, :])
                    if h == 3 or h == 7:
                        g = h // 4
                        rs32 = cpool.tile([128, L], F32, tag="rs32", bufs=1,
                                          name=f"rs32_{g}")
                        nc.vector.reciprocal_approx_fast(
                            out=rs32[:], in_=sums_g[g][:])
                        nc.vector.tensor_copy(rs_g[g][:], rs32[:])
                        # normalize this group's ytr in C's engine slack
                        for fbn in (2 * g, 2 * g + 1):
                            sel = sel_c[fbn % 2]
                            for qn in range(4):
                                nps = cps.tile([128, 512], F32, tag="bc2",
                                               bufs=2)
                                nc.tensor.matmul(
                                    nps[:],
                                    sel[:],
                                    rs_g[g][:, qn * 512:(qn + 1) * 512],
                                    start=True, stop=True)
                                nc.vector.tensor_mul(
                                    ytr[fbn][:, qn * 512:(qn + 1) * 512],
                                    ytr[fbn][:, qn * 512:(qn + 1) * 512],
                                    nps[:])

            # ---- Phase D: normalize + output projection ----
            with ExitStack() as dctx:
                dpool = dctx.enter_context(tc.tile_pool(name="out_sb", bufs=1))
                dps = dctx.enter_context(
                    tc.tile_pool(name="out_ps", bufs=1, space="PSUM"))
                wo_sb = []
                for fc in range(4):
                    w = dpool.tile([128, D], BF16, tag=f"wo{fc}")
                    nc.sync.dma_start(w[:], wout[fc * 128:(fc + 1) * 128, :])
                    wo_sb.append(w)
                for qn in range(4):
                    for nb in range(8):
                        ps = dps.tile([128, 512], F32, tag="oproj", bufs=3)
                        for fc in range(4):
                            nc.tensor.matmul(
                                ps[:],
                                wo_sb[fc][:, nb * 128:(nb + 1) * 128],
                                ytr[fc][:, qn * 512:(qn + 1) * 512],
                                start=(fc == 0), stop=(fc == 3))
                        ot = dpool.tile([128, 512], F32, tag="ot", bufs=3)
                        nc.scalar.activation(ot[:], ps[:], COPY)
                        nc.sync.dma_start(
                            outT[nb * 128:(nb + 1) * 128,
                                 qn * 512:(qn + 1) * 512], ot[:])
    nc.compile()
    return nc


def get_nc():
    global _NC
    if _NC is None:
        _NC = _build_program()
    return _NC


# --------------------------------------------------------------------------- #
# Host side
# --------------------------------------------------------------------------- #

def _rope_tables(pos, g):
    """Feature-major folded RoPE(+gain) tables, replicated for a 2-head tile."""
    pos = np.asarray(pos).astype(np.float32)
    g = np.asarray(g, dtype=np.float32)
    inv = (1.0 / (10000.0 ** (np.arange(0, DH, 2, dtype=np.float32)
                              / np.float32(DH)))).astype(np.float32)
    ang = pos[:, None] * inv[None, :]                      # (L, 32)
    cos, sin = np.cos(ang, dtype=np.float32), np.sin(ang, dtype=np.float32)
    j = np.arange(DH)
    C = (g[j][:, None] * cos[:, j % 32].T).astype(np.float32)       # (64, L)
    sign = np.where(j < 32, -1.0, 1.0).astype(np.float32)
    S = (sign[:, None] * g[(j + 32) % 64][:, None]
         * sin[:, j % 32].T).astype(np.float32)
    BF = ml_dtypes.bfloat16
    return (np.ascontiguousarray(np.tile(C, (2, 1))).astype(BF),
            np.ascontiguousarray(np.tile(S, (2, 1))).astype(BF))  # (128, L)


def _rot_cols(w):
    """Swap the two 32-col halves of each 64-col head block: W @ P^T."""
    w3 = w.reshape(D, HC, 2, 32)
    return np.ascontiguousarray(w3[:, :, ::-1, :].reshape(D, F))


def make_in_maps(queries, kv, Wq, Wkv, Wout, g_q, g_k, pos_q, pos_k):
    BF = ml_dtypes.bfloat16
    queries = np.asarray(queries, dtype=np.float32)
    kv = np.asarray(kv, dtype=np.float32)
    Wq = np.asarray(Wq, dtype=np.float32)
    Wkv = np.asarray(Wkv, dtype=np.float32)
    Wout = np.asarray(Wout, dtype=np.float32)

    cq, sq = _rope_tables(pos_q, g_q)
    ck, sk = _rope_tables(pos_k, g_k)
    bdiag = np.zeros((128, 2), BF)
    bdiag[0:64, 0] = 1.0
    bdiag[64:128, 1] = 1.0
    bmap = np.zeros((2, 128), BF)
    bmap[0, 0:64] = 1.0
    bmap[1, 64:128] = 1.0
    selA = np.zeros((128, 128), BF)
    selA[0, 0:64] = 1.0
    selA[32, 64:128] = 1.0
    selB = np.zeros((128, 128), BF)
    selB[64, 0:64] = 1.0
    selB[96, 64:128] = 1.0

    Wkv3 = Wkv.reshape(D, 16, 2 * DH)
    xqT_b = [np.ascontiguousarray(queries[b].T.astype(BF)) for b in range(4)]
    xkvT_b = [np.ascontiguousarray(kv[b].T.astype(BF)) for b in range(4)]
    in_maps = []
    for c in range(N_CORES):
        b, grp = c // 2, c % 2
        hs = slice(grp * HC, (grp + 1) * HC)
        wq_g = Wq[:, grp * F:(grp + 1) * F]
        wk_g = Wkv3[:, hs, :DH].reshape(D, F)
        in_maps.append({
            "xqT": xqT_b[b],
            "xkvT": xkvT_b[b],
            "wq": np.ascontiguousarray(wq_g.astype(BF)),
            "wqr": _rot_cols(wq_g).astype(BF),
            "wk": np.ascontiguousarray(wk_g.astype(BF)),
            "wkr": _rot_cols(wk_g).astype(BF),
            "wv": np.ascontiguousarray(
                Wkv3[:, hs, DH:].reshape(D, F).astype(BF)),
            "wout": np.ascontiguousarray(
                Wout[grp * F:(grp + 1) * F, :].astype(BF)),
            "cq": cq, "sq": sq, "ck": ck, "sk": sk,
            "bdiag": bdiag, "bmap": bmap, "selA": selA, "selB": selB,
        })
    return in_maps


def kernel(queries, kv, Wq, Wkv, Wout, g_q, g_k, pos_q, pos_k):
    global LAST_RESULTS
    nc = get_nc()
    in_maps = make_in_maps(queries, kv, Wq, Wkv, Wout, g_q, g_k, pos_q, pos_k)
    trace = bool(int(os.environ.get("KERNEL_TRACE", "0")))
    kw = {}
    if trace:
        kw["tmpdir"] = os.environ.get("KERNEL_TRACE_DIR") or None
    res = run_bass_kernel_spmd(nc, in_maps, core_ids=list(range(N_CORES)),
                               trace=trace, **kw)
    LAST_RESULTS = res
    out = np.empty((4, L, D), np.float32)
    for b in range(4):
        out[b] = (res.results[2 * b]["outT"]
                  + res.results[2 * b + 1]["outT"]).T
    return out


# revision 22
# speedup vs baseline: 1.0997x; 1.0997x over previous
"""Trainium2 Bass kernel for nn_CrossAttention (B=4, Lq=Lk=2048, D=1024, H=16, d=64).

Sharding: 8 cores = 4 batches x 2 head-groups (8 heads each).
Each core computes a partial out^T = Wout_g^T @ y_g^T for its (batch, head-group);
host sums the two head-group partials per batch and transposes.

All matmul operands are bf16 (fp32 PSUM accumulation). Device layout is
feature-major ("T" = [feature, seq]):
  qT/kT: [512, L] bf16 (8 heads x 64 dims on partitions, seq on free axis)
  S^T:   [k, q] tiles -> softmax sum via an appended ones-column in v (M=65)
  exp:   ACT, with the k-side RMSNorm rstd (and the 1/sqrt(d) scale) folded
         into the per-partition activation scale operand.
RoPE rotate-half comes from a second projection against host-permuted weights
(Wq_rot/Wk_rot); the C/S combine runs on the vector engine straight out of
PSUM. The q projection runs fb-outer so qT[0] completes early and attention
matmuls chase the projection without a pipeline bubble.
"""
import os
import numpy as np
from contextlib import ExitStack

import ml_dtypes

import concourse.bass as bass
import concourse.tile as tile
from concourse import bacc, mybir
from concourse.bass_utils import run_bass_kernel_spmd

F32 = mybir.dt.float32
BF16 = mybir.dt.bfloat16
EXP = mybir.ActivationFunctionType.Exp
SQUARE = mybir.ActivationFunctionType.Square
SQRT = mybir.ActivationFunctionType.Sqrt
COPY = mybir.ActivationFunctionType.Copy

D = 1024          # model dim
L = 2048          # seq len (q and k)
HC = 8            # heads per core
DH = 64           # head dim
F = HC * DH       # 512 local features
N_CORES = 8
EPS = float(np.finfo(np.float32).eps)

LAST_RESULTS = None  # BassKernelResults of the most recent run (for test harness)
_NC = None


# --------------------------------------------------------------------------- #
# Device program
# --------------------------------------------------------------------------- #

def _norm_rope_chunk(tc, pool, pps, dst, ps, rot, c_sb, s_sb, bdiag, bmap,
                     eps_t, side, fb, col0, rk_dram):
    """RMSNorm + RoPE for one projected [128, 512] chunk, out of PSUM."""
    nc = tc.nc
    # sum of squares over each head's 64 partition rows
    sq = pool.tile([128, 512], BF16, tag="sq", bufs=2)
    nc.scalar.activation(sq[:], ps[:], SQUARE)
    vps = pps.tile([2, 512], F32, tag="var", bufs=2)
    nc.tensor.matmul(vps[:], bdiag[:], sq[:], start=True, stop=True)
    std = pool.tile([2, 512], F32, tag="std", bufs=2)
    rstd = pool.tile([2, 512], F32, tag="rstd", bufs=2)
    if side == "q":
        # std = sqrt(raw/64 + eps); rstd = 1/std
        nc.scalar.activation(std[:], vps[:], SQRT,
                             bias=eps_t[:], scale=1.0 / 64.0)
        nc.vector.reciprocal_approx_fast(out=rstd[:], in_=std[:])
        rstd_r = pool.tile([2, 512], BF16, tag="rstdr", bufs=2)
        nc.vector.tensor_copy(rstd_r[:], rstd[:])
        bps = pps.tile([128, 512], F32, tag="bc", bufs=2)
        nc.tensor.matmul(bps[:], bmap[:], rstd_r[:], start=True, stop=True)
    else:
        # fold the 1/8 attention scale: rk = 1/(8*std) = 1/sqrt(64*(raw/64+eps))
        nc.scalar.activation(std[:], vps[:], SQRT, bias=eps_t[:], scale=1.0)
        nc.vector.reciprocal_approx_fast(out=rstd[:], in_=std[:])
        nc.sync.dma_start(
            rk_dram[2 * fb:2 * fb + 2, col0:col0 + 512], rstd[:])
    # rope combine on the vector engine, reading PSUM directly
    t2 = pool.tile([128, 512], BF16, tag="t2", bufs=2)
    nc.vector.tensor_mul(t2[:], ps[:], c_sb[:, col0:col0 + 512])
    tmp = pool.tile([128, 512], BF16, tag="tmp", bufs=2)
    nc.vector.tensor_mul(tmp[:], rot[:], s_sb[:, col0:col0 + 512])
    chunk = dst[fb][:, col0:col0 + 512]
    if side == "q":
        t3 = pool.tile([128, 512], BF16, tag="t3", bufs=2)
        nc.vector.tensor_add(t3[:], t2[:], tmp[:])
        nc.vector.tensor_mul(chunk, t3[:], bps[:])
    else:
        nc.vector.tensor_add(chunk, t2[:], tmp[:])


def _proj_pair(nc, pps, w_sb, wr_sb, x_ap_chunks):
    """Dual projection (plain + rotated weights) of one 512-col chunk."""
    ps = pps.tile([128, 512], F32, tag="proj", bufs=2)
    for dc in range(8):
        nc.tensor.matmul(ps[:], w_sb[dc], x_ap_chunks[dc],
                         start=(dc == 0), stop=(dc == 7))
    rot = pps.tile([128, 512], F32, tag="rot", bufs=2)
    for dc in range(8):
        nc.tensor.matmul(rot[:], wr_sb[dc], x_ap_chunks[dc],
                         start=(dc == 0), stop=(dc == 7))
    return ps, rot


def _build_program():
    nc = bacc.Bacc("TRN2", target_bir_lowering=False, debug=False,
                   num_devices=N_CORES)
    dt = nc.dram_tensor
    xqT = dt("xqT", (D, L), BF16, kind="ExternalInput").ap()
    xkvT = dt("xkvT", (D, L), BF16, kind="ExternalInput").ap()
    wq = dt("wq", (D, F), BF16, kind="ExternalInput").ap()
    wqr = dt("wqr", (D, F), BF16, kind="ExternalInput").ap()
    wk = dt("wk", (D, F), BF16, kind="ExternalInput").ap()
    wkr = dt("wkr", (D, F), BF16, kind="ExternalInput").ap()
    wv = dt("wv", (D, F), BF16, kind="ExternalInput").ap()
    wout = dt("wout", (F, D), BF16, kind="ExternalInput").ap()
    cq = dt("cq", (128, L), BF16, kind="ExternalInput").ap()
    sq_t = dt("sq", (128, L), BF16, kind="ExternalInput").ap()
    ck = dt("ck", (128, L), BF16, kind="ExternalInput").ap()
    sk_t = dt("sk", (128, L), BF16, kind="ExternalInput").ap()
    bdiag_d = dt("bdiag", (128, 2), BF16, kind="ExternalInput").ap()
    bmap_d = dt("bmap", (2, 128), BF16, kind="ExternalInput").ap()
    selA_d = dt("selA", (128, 128), BF16, kind="ExternalInput").ap()
    selB_d = dt("selB", (128, 128), BF16, kind="ExternalInput").ap()
    outT = dt("outT", (D, L), BF16, kind="ExternalOutput").ap()

    with tile.TileContext(nc) as tc:
        with ExitStack() as ctx:
            big = ctx.enter_context(tc.tile_pool(name="big", bufs=1))
            dram = ctx.enter_context(tc.tile_pool(name="dram", bufs=1, space="DRAM"))

            kT = [big.tile([128, L], BF16, tag=f"kT{i}", name=f"kT{i}") for i in range(4)]
            qT = [big.tile([128, L], BF16, tag=f"qT{i}", name=f"qT{i}") for i in range(4)]
            vaug = [big.tile([128, HC * 65], BF16, tag=f"v{i}", name=f"vaug{i}") for i in range(16)]
            rk_dram = dram.tile([HC, L], F32, tag="rk")

            bdiag = big.tile([128, 2], BF16, tag="bdiag")
            nc.sync.dma_start(bdiag[:], bdiag_d[:])
            bmap = big.tile([2, 128], BF16, tag="bmap")
            nc.sync.dma_start(bmap[:], bmap_d[:])

            # first DMAs in the queue: the xkv half phase A starts on
            xpool = ctx.enter_context(tc.tile_pool(name="xp", bufs=1))
            xkv_h0 = []
            for dc in range(8):
                x = xpool.tile([128, 1024], BF16, tag=f"xkv{dc}", bufs=1)
                nc.sync.dma_start(x[:], xkvT[dc * 128:(dc + 1) * 128, 0:1024])
                xkv_h0.append(x)

            # weights [128, F] x8 per projection; phase-A inputs load first
            def load_w(w_dram, name):
                tiles = []
                for dc in range(8):
                    w = big.tile([128, F], BF16, tag=f"{name}{dc}",
                                 name=f"{name}{dc}")
                    nc.sync.dma_start(w[:], w_dram[dc * 128:(dc + 1) * 128, :])
                    tiles.append(w)
                return tiles
            wk_sb = load_w(wk, "wk")
            wkr_sb = load_w(wkr, "wkr")
            wv_sb = load_w(wv, "wv")
            ck_sb = big.tile([128, L], BF16, tag="ckt")
            nc.sync.dma_start(ck_sb[:], ck[:])
            sk_sb = big.tile([128, L], BF16, tag="skt")
            nc.sync.dma_start(sk_sb[:], sk_t[:])
            # phase-B inputs queue behind phase A's (prefetch during A)
            wq_sb = load_w(wq, "wq")
            wqr_sb = load_w(wqr, "wqr")
            cq_sb = big.tile([128, L], BF16, tag="cqt")
            nc.sync.dma_start(cq_sb[:], cq[:])
            sq_sb = big.tile([128, L], BF16, tag="sqt")
            nc.sync.dma_start(sq_sb[:], sq_t[:])
            xq_sb = []
            for dc in range(8):
                x = big.tile([128, L], BF16, tag=f"xq{dc}", name=f"xq{dc}")
                nc.sync.dma_start(x[:], xqT[dc * 128:(dc + 1) * 128, :])
                xq_sb.append(x)

            # ---- Phase A: k/v projections (xkv by seq halves) ----
            with ExitStack() as actx:
                apool = actx.enter_context(tc.tile_pool(name="a_sb", bufs=1))
                aps = actx.enter_context(tc.tile_pool(name="a_ps", bufs=1,
                                                      space="PSUM"))
                eps_k = apool.tile([2, 1], F32, tag="epsk")
                nc.gpsimd.memset(eps_k[:], 64.0 * EPS)
                for lh in range(2):
                    if lh == 0:
                        xkv_h = xkv_h0
                    else:
                        xkv_h = []
                        for dc in range(8):
                            x = xpool.tile([128, 1024], BF16, tag=f"xkv{dc}",
                                           bufs=1)
                            nc.sync.dma_start(
                                x[:], xkvT[dc * 128:(dc + 1) * 128,
                                           1024:2048])
                            xkv_h.append(x)
                    for fb in range(4):
                        for qn in range(2):
                            col0 = lh * 1024 + qn * 512
                            ps, rot = _proj_pair(
                                nc, aps,
                                [wk_sb[dc][:, fb * 128:(fb + 1) * 128]
                                 for dc in range(8)],
                                [wkr_sb[dc][:, fb * 128:(fb + 1) * 128]
                                 for dc in range(8)],
                                [xkv_h[dc][:, qn * 512:(qn + 1) * 512]
                                 for dc in range(8)])
                            _norm_rope_chunk(tc, apool, aps, kT, ps, rot,
                                             ck_sb, sk_sb, bdiag, bmap,
                                             eps_k, "k", fb, col0, rk_dram)
                    for lc in range(8):
                        kc = lh * 8 + lc
                        ps = aps.tile([128, 512], F32, tag="proj", bufs=2)
                        for dc in range(8):
                            nc.tensor.matmul(
                                ps[:],
                                xkv_h[dc][:, lc * 128:(lc + 1) * 128],
                                wv_sb[dc][:],
                                start=(dc == 0), stop=(dc == 7))
                        va = vaug[kc]
                        nc.gpsimd.memset(va[:], 1.0)
                        va3 = va.rearrange("p (h c) -> p h c", c=65)
                        ps3 = ps.rearrange("p (h c) -> p h c", c=64)
                        nc.vector.tensor_copy(va3[:, :, 0:64], ps3[:])

            # ---- Phase B: q projection, fb-outer so qT[0] finishes first ----
            with ExitStack() as bctx:
                bpool = bctx.enter_context(tc.tile_pool(name="b_sb", bufs=1))
                bps_p = bctx.enter_context(tc.tile_pool(name="b_ps", bufs=1,
                                                        space="PSUM"))
                eps_q = bpool.tile([2, 1], F32, tag="epsq")
                nc.gpsimd.memset(eps_q[:], EPS)
                for fb in range(4):
                    for cn in range(4):   # 512-col chunks across full L
                        col0 = cn * 512
                        ps, rot = _proj_pair(
                            nc, bps_p,
                            [wq_sb[dc][:, fb * 128:(fb + 1) * 128]
                             for dc in range(8)],
                            [wqr_sb[dc][:, fb * 128:(fb + 1) * 128]
                             for dc in range(8)],
                            [xq_sb[dc][:, col0:col0 + 512]
                             for dc in range(8)])
                        _norm_rope_chunk(tc, bpool, bps_p, qT, ps, rot,
                                         cq_sb, sq_sb, bdiag, bmap,
                                         eps_q, "q", fb, col0, None)

            # ---- Phases C+D persistents ----
            p2 = ctx.enter_context(tc.tile_pool(name="p2", bufs=1))
            ytr = [p2.tile([128, L], BF16, tag=f"ytr{i}", name=f"ytr{i}")
                   for i in range(4)]
            sums_g = [p2.tile([128, L], F32, tag=f"sums{g}", name=f"sums{g}")
                      for g in range(2)]
            nc.gpsimd.memset(sums_g[0][:], 1.0)
            nc.gpsimd.memset(sums_g[1][:], 1.0)
            rs_g = [p2.tile([128, L], BF16, tag=f"rs{g}", name=f"rs{g}")
                    for g in range(2)]

            # ---- Phase C: attention (2 q-passes per head) ----
            with ExitStack() as cctx:
                cpool = cctx.enter_context(tc.tile_pool(name="att_sb", bufs=1))
                cps = cctx.enter_context(
                    tc.tile_pool(name="att_ps", bufs=1, space="PSUM"))
                rk_sb = cpool.tile([128, HC, 16], F32, tag="rk")
                nc.sync.dma_start(
                    rk_sb[:], rk_dram.rearrange("h (kc p) -> p h kc", p=128))
                for h in range(HC):
                    fb, off = h // 2, (h % 2) * 64
                    for pss in range(2):
                        yps = [cps.tile([65, 512], F32, tag=f"y{j}", bufs=1,
                                        name=f"yps{h}_{pss}_{j}")
                               for j in range(2)]
                        for kc in range(16):
                            va3 = vaug[kc].rearrange("p (h c) -> p h c", c=65)
                            rk_ap = rk_sb[:, h, kc:kc + 1]
                            sps = cps.tile([128, 1024], F32, tag="s", bufs=3)
                            for j in range(2):
                                qn = pss * 2 + j
                                nc.tensor.matmul(
                                    sps[:, j * 512:(j + 1) * 512],
                                    kT[fb][off:off + 64,
                                           kc * 128:(kc + 1) * 128],
                                    qT[fb][off:off + 64,
                                           qn * 512:(qn + 1) * 512],
                                    start=True, stop=True)
                            pt = cpool.tile([128, 1024], BF16, tag="p", bufs=3)
                            nc.scalar.activation(pt[:], sps[:], EXP,
                                                 scale=rk_ap)
                            for j in range(2):
                                nc.tensor.matmul(
                                    yps[j][:], va3[:, h, :],
                                    pt[:, j * 512:(j + 1) * 512],
                                    start=(kc == 0), stop=(kc == 15))
                        for j in range(2):
                            qn = pss * 2 + j
                            nc.vector.tensor_copy(
                                ytr[fb][off:off + 64,
                                        qn * 512:(qn + 1) * 512],
                                yps[j][0:64, :])
                            slot = 32 * (h % 4)
                            nc.vector.tensor_copy(
                                sums_g[h // 4][slot:slot + 1,
                                               qn * 512:(qn + 1) * 512],
                                yps[j][64:65, :])
                    if h == 3 or h == 7:
                        g = h // 4
                        rs32 = cpool.tile([128, L], F32, tag="rs32", bufs=1,
                                          name=f"rs32_{g}")
                        nc.vector.reciprocal_approx_fast(
                            out=rs32[:], in_=sums_g[g][:])
                        nc.vector.tensor_copy(rs_g[g][:], rs32[:])

            # ---- Phase D: normalize + output projection ----
            with ExitStack() as dctx:
                dpool = dctx.enter_context(tc.tile_pool(name="out_sb", bufs=1))
                dps = dctx.enter_context(
                    tc.tile_pool(name="out_ps", bufs=1, space="PSUM"))
                wo_sb = []
                for fc in range(4):
                    w = dpool.tile([128, D], BF16, tag=f"wo{fc}")
                    nc.sync.dma_start(w[:], wout[fc * 128:(fc + 1) * 128, :])
                    wo_sb.append(w)
                sel_sb = []
                for i, sd in enumerate((selA_d, selB_d)):
                    s = dpool.tile([128, 128], BF16, tag=f"sel{i}",
                                   name=f"sel{i}")
                    nc.sync.dma_start(s[:], sd[:])
                    sel_sb.append(s)
                for qn in range(4):
                    for fb in range(4):
                        bps = dps.tile([128, 512], F32, tag="bc2", bufs=2)
                        nc.tensor.matmul(
                            bps[:],
                            sel_sb[fb % 2][:],
                            rs_g[fb // 2][:, qn * 512:(qn + 1) * 512],
                            start=True, stop=True)
                        nc.vector.tensor_mul(
                            ytr[fb][:, qn * 512:(qn + 1) * 512],
                            ytr[fb][:, qn * 512:(qn + 1) * 512], bps[:])
                    for nb in range(8):
                        ps = dps.tile([128, 512], F32, tag="oproj", bufs=3)
                        for fc in range(4):
                            nc.tensor.matmul(
                                ps[:],
                                wo_sb[fc][:, nb * 128:(nb + 1) * 128],
                                ytr[fc][:, qn * 512:(qn + 1) * 512],
                                start=(fc == 0), stop=(fc == 3))
                        ot = dpool.tile([128, 512], BF16, tag="ot", bufs=3)
                        nc.scalar.activation(ot[:], ps[:], COPY)
                        nc.sync.dma_start(
                            outT[nb * 128:(nb + 1) * 128,
                                 qn * 512:(qn + 1) * 512], ot[:])
    nc.compile()
    return nc


def get_nc():
    global _NC
    if _NC is None:
        _NC = _build_program()
    return _NC


# --------------------------------------------------------------------------- #
# Host side
# --------------------------------------------------------------------------- #

def _rope_tables(pos, g):
    """Feature-major folded RoPE(+gain) tables, replicated for a 2-head tile."""
    pos = np.asarray(pos).astype(np.float32)
    g = np.asarray(g, dtype=np.float32)
    inv = (1.0 / (10000.0 ** (np.arange(0, DH, 2, dtype=np.float32)
                              / np.float32(DH)))).astype(np.float32)
    ang = pos[:, None] * inv[None, :]                      # (L, 32)
    cos, sin = np.cos(ang, dtype=np.float32), np.sin(ang, dtype=np.float32)
    j = np.arange(DH)
    C = (g[j][:, None] * cos[:, j % 32].T).astype(np.float32)       # (64, L)
    sign = np.where(j < 32, -1.0, 1.0).astype(np.float32)
    S = (sign[:, None] * g[(j + 32) % 64][:, None]
         * sin[:, j % 32].T).astype(np.float32)
    BF = ml_dtypes.bfloat16
    return (np.ascontiguousarray(np.tile(C, (2, 1))).astype(BF),
            np.ascontiguousarray(np.tile(S, (2, 1))).astype(BF))  # (128, L)


def _rot_cols(w):
    """Swap the two 32-col halves of each 64-col head block: W @ P^T."""
    w3 = w.reshape(D, HC, 2, 32)
    return np.ascontiguousarray(w3[:, :, ::-1, :].reshape(D, F))


def make_in_maps(queries, kv, Wq, Wkv, Wout, g_q, g_k, pos_q, pos_k):
    BF = ml_dtypes.bfloat16
    queries = np.asarray(queries, dtype=np.float32)
    kv = np.asarray(kv, dtype=np.float32)
    Wq = np.asarray(Wq, dtype=np.float32)
    Wkv = np.asarray(Wkv, dtype=np.float32)
    Wout = np.asarray(Wout, dtype=np.float32)

    cq, sq = _rope_tables(pos_q, g_q)
    ck, sk = _rope_tables(pos_k, g_k)
    bdiag = np.zeros((128, 2), BF)
    bdiag[0:64, 0] = 1.0
    bdiag[64:128, 1] = 1.0
    bmap = np.zeros((2, 128), BF)
    bmap[0, 0:64] = 1.0
    bmap[1, 64:128] = 1.0
    selA = np.zeros((128, 128), BF)
    selA[0, 0:64] = 1.0
    selA[32, 64:128] = 1.0
    selB = np.zeros((128, 128), BF)
    selB[64, 0:64] = 1.0
    selB[96, 64:128] = 1.0

    Wkv3 = Wkv.reshape(D, 16, 2 * DH)
    xqT_b = [np.ascontiguousarray(queries[b].T.astype(BF)) for b in range(4)]
    xkvT_b = [np.ascontiguousarray(kv[b].T.astype(BF)) for b in range(4)]
    in_maps = []
    for c in range(N_CORES):
        b, grp = c // 2, c % 2
        hs = slice(grp * HC, (grp + 1) * HC)
        wq_g = Wq[:, grp * F:(grp + 1) * F]
        wk_g = Wkv3[:, hs, :DH].reshape(D, F)
        in_maps.append({
            "xqT": xqT_b[b],
            "xkvT": xkvT_b[b],
            "wq": np.ascontiguousarray(wq_g.astype(BF)),
            "wqr": _rot_cols(wq_g).astype(BF),
            "wk": np.ascontiguousarray(wk_g.astype(BF)),
            "wkr": _rot_cols(wk_g).astype(BF),
            "wv": np.ascontiguousarray(
                Wkv3[:, hs, DH:].reshape(D, F).astype(BF)),
            "wout": np.ascontiguousarray(
                Wout[grp * F:(grp + 1) * F, :].astype(BF)),
            "cq": cq, "sq": sq, "ck": ck, "sk": sk,
            "bdiag": bdiag, "bmap": bmap, "selA": selA, "selB": selB,
        })
    return in_maps


def kernel(queries, kv, Wq, Wkv, Wout, g_q, g_k, pos_q, pos_k):
    global LAST_RESULTS
    nc = get_nc()
    in_maps = make_in_maps(queries, kv, Wq, Wkv, Wout, g_q, g_k, pos_q, pos_k)
    trace = bool(int(os.environ.get("KERNEL_TRACE", "0")))
    kw = {}
    if trace:
        kw["tmpdir"] = os.environ.get("KERNEL_TRACE_DIR") or None
    res = run_bass_kernel_spmd(nc, in_maps, core_ids=list(range(N_CORES)),
                               trace=trace, **kw)
    LAST_RESULTS = res
    out = np.empty((4, L, D), np.float32)
    for b in range(4):
        out[b] = (res.results[2 * b]["outT"].astype(np.float32)
                  + res.results[2 * b + 1]["outT"].astype(np.float32)).T
    return out


# revision 27
# speedup vs baseline: 1.1051x; 1.0049x over previous
"""Trainium2 Bass kernel for nn_CrossAttention (B=4, Lq=Lk=2048, D=1024, H=16, d=64).

Sharding: 8 cores = 4 batches x 2 head-groups (8 heads each).
Each core computes a partial out^T = Wout_g^T @ y_g^T for its (batch, head-group);
host sums the two head-group partials per batch and transposes.

All matmul operands are bf16 (fp32 PSUM accumulation). Device layout is
feature-major ("T" = [feature, seq]):
  qT/kT: [512, L] bf16 (8 heads x 64 dims on partitions, seq on free axis)
  S^T:   [k, q] tiles -> softmax sum via an appended ones-column in v (M=65)
  exp:   ACT, with the k-side RMSNorm rstd (and the 1/sqrt(d) scale) folded
         into the per-partition activation scale operand.
RoPE rotate-half comes from a second projection against host-permuted weights
(Wq_rot/Wk_rot); the C/S combine runs on the vector engine straight out of
PSUM. The q projection runs fb-outer so qT[0] completes early and attention
matmuls chase the projection without a pipeline bubble.
"""
import os
import numpy as np
from contextlib import ExitStack

import ml_dtypes

import concourse.bass as bass
import concourse.tile as tile
from concourse import bacc, mybir
from concourse.bass_utils import run_bass_kernel_spmd

F32 = mybir.dt.float32
BF16 = mybir.dt.bfloat16
EXP = mybir.ActivationFunctionType.Exp
SQUARE = mybir.ActivationFunctionType.Square
SQRT = mybir.ActivationFunctionType.Sqrt
COPY = mybir.ActivationFunctionType.Copy

D = 1024          # model dim
L = 2048          # seq len (q and k)
HC = 8            # heads per core
DH = 64           # head dim
F = HC * DH       # 512 local features
N_CORES = 8
EPS = float(np.finfo(np.float32).eps)

LAST_RESULTS = None  # BassKernelResults of the most recent run (for test harness)
_NC = None


# --------------------------------------------------------------------------- #
# Device program
# --------------------------------------------------------------------------- #

def _norm_rope_chunk(tc, pool, pps, dst, ps, rot, c_sb, s_sb, bdiag, bmap,
                     eps_t, side, fb, col0, rk_dram):
    """RMSNorm + RoPE for one projected [128, 512] chunk, out of PSUM."""
    nc = tc.nc
    # sum of squares over each head's 64 partition rows
    sq = pool.tile([128, 512], BF16, tag="sq", bufs=2)
    nc.scalar.activation(sq[:], ps[:], SQUARE)
    vps = pps.tile([2, 512], F32, tag="var", bufs=2)
    nc.tensor.matmul(vps[:], bdiag[:], sq[:], start=True, stop=True)
    std = pool.tile([2, 512], F32, tag="std", bufs=2)
    rstd = pool.tile([2, 512], F32, tag="rstd", bufs=2)
    if side == "q":
        # std = sqrt(raw/64 + eps); rstd = 1/std
        nc.scalar.activation(std[:], vps[:], SQRT,
                             bias=eps_t[:], scale=1.0 / 64.0)
        nc.vector.reciprocal_approx_fast(out=rstd[:], in_=std[:])
        rstd_r = pool.tile([2, 512], BF16, tag="rstdr", bufs=2)
        nc.vector.tensor_copy(rstd_r[:], rstd[:])
        bps = pps.tile([128, 512], F32, tag="bc", bufs=2)
        nc.tensor.matmul(bps[:], bmap[:], rstd_r[:], start=True, stop=True)
    else:
        # fold the 1/8 attention scale: rk = 1/(8*std) = 1/sqrt(64*(raw/64+eps))
        nc.scalar.activation(std[:], vps[:], SQRT, bias=eps_t[:], scale=1.0)
        nc.vector.reciprocal_approx_fast(out=rstd[:], in_=std[:])
        nc.sync.dma_start(
            rk_dram[2 * fb:2 * fb + 2, col0:col0 + 512], rstd[:])
    # rope combine on the vector engine, reading PSUM directly
    t2 = pool.tile([128, 512], BF16, tag="t2", bufs=2)
    nc.vector.tensor_mul(t2[:], ps[:], c_sb[:, col0:col0 + 512])
    tmp = pool.tile([128, 512], BF16, tag="tmp", bufs=2)
    nc.vector.tensor_mul(tmp[:], rot[:], s_sb[:, col0:col0 + 512])
    chunk = dst[fb][:, col0:col0 + 512]
    if side == "q":
        t3 = pool.tile([128, 512], BF16, tag="t3", bufs=2)
        nc.vector.tensor_add(t3[:], t2[:], tmp[:])
        nc.vector.tensor_mul(chunk, t3[:], bps[:])
    else:
        nc.vector.tensor_add(chunk, t2[:], tmp[:])


def _proj_pair(nc, pps, w_sb, wr_sb, x_ap_chunks):
    """Dual projection (plain + rotated weights) of one 512-col chunk."""
    ps = pps.tile([128, 512], F32, tag="proj", bufs=2)
    for dc in range(8):
        nc.tensor.matmul(ps[:], w_sb[dc], x_ap_chunks[dc],
                         start=(dc == 0), stop=(dc == 7))
    rot = pps.tile([128, 512], F32, tag="rot", bufs=2)
    for dc in range(8):
        nc.tensor.matmul(rot[:], wr_sb[dc], x_ap_chunks[dc],
                         start=(dc == 0), stop=(dc == 7))
    return ps, rot


def _build_program():
    nc = bacc.Bacc("TRN2", target_bir_lowering=False, debug=False,
                   num_devices=N_CORES)
    dt = nc.dram_tensor
    xqT = dt("xqT", (D, L), BF16, kind="ExternalInput").ap()
    xkvT = dt("xkvT", (D, L), BF16, kind="ExternalInput").ap()
    wq = dt("wq", (D, F), BF16, kind="ExternalInput").ap()
    wqr = dt("wqr", (D, F), BF16, kind="ExternalInput").ap()
    wk = dt("wk", (D, F), BF16, kind="ExternalInput").ap()
    wkr = dt("wkr", (D, F), BF16, kind="ExternalInput").ap()
    wv = dt("wv", (D, F), BF16, kind="ExternalInput").ap()
    wout = dt("wout", (F, D), BF16, kind="ExternalInput").ap()
    cq = dt("cq", (128, L), BF16, kind="ExternalInput").ap()
    sq_t = dt("sq", (128, L), BF16, kind="ExternalInput").ap()
    ck = dt("ck", (128, L), BF16, kind="ExternalInput").ap()
    sk_t = dt("sk", (128, L), BF16, kind="ExternalInput").ap()
    bdiag_d = dt("bdiag", (128, 2), BF16, kind="ExternalInput").ap()
    bmap_d = dt("bmap", (2, 128), BF16, kind="ExternalInput").ap()
    selA_d = dt("selA", (128, 128), BF16, kind="ExternalInput").ap()
    selB_d = dt("selB", (128, 128), BF16, kind="ExternalInput").ap()
    outT = dt("outT", (D, L), BF16, kind="ExternalOutput").ap()

    with tile.TileContext(nc) as tc:
        with ExitStack() as ctx:
            big = ctx.enter_context(tc.tile_pool(name="big", bufs=1))
            dram = ctx.enter_context(tc.tile_pool(name="dram", bufs=1, space="DRAM"))

            kT = [big.tile([128, L], BF16, tag=f"kT{i}", name=f"kT{i}") for i in range(4)]
            qT = [big.tile([128, L], BF16, tag=f"qT{i}", name=f"qT{i}") for i in range(4)]
            vaug = [big.tile([128, HC * 65], BF16, tag=f"v{i}", name=f"vaug{i}") for i in range(16)]
            rk_dram = dram.tile([HC, L], F32, tag="rk")

            bdiag = big.tile([128, 2], BF16, tag="bdiag")
            nc.sync.dma_start(bdiag[:], bdiag_d[:])
            bmap = big.tile([2, 128], BF16, tag="bmap")
            nc.sync.dma_start(bmap[:], bmap_d[:])

            # first DMAs in the queue: the xkv half phase A starts on
            xpool = ctx.enter_context(tc.tile_pool(name="xp", bufs=1))
            xkv_h0 = []
            for dc in range(8):
                x = xpool.tile([128, 1024], BF16, tag=f"xkv{dc}", bufs=1)
                nc.sync.dma_start(x[:, 0:512],
                                  xkvT[dc * 128:(dc + 1) * 128, 0:512])
                xkv_h0.append(x)
            for dc in range(8):
                nc.sync.dma_start(xkv_h0[dc][:, 512:1024],
                                  xkvT[dc * 128:(dc + 1) * 128, 512:1024])

            # weights [128, F] x8 per projection; phase-A inputs load first
            def load_w(w_dram, name):
                tiles = []
                for dc in range(8):
                    w = big.tile([128, F], BF16, tag=f"{name}{dc}",
                                 name=f"{name}{dc}")
                    nc.sync.dma_start(w[:], w_dram[dc * 128:(dc + 1) * 128, :])
                    tiles.append(w)
                return tiles
            wk_sb = load_w(wk, "wk")
            wkr_sb = load_w(wkr, "wkr")
            wv_sb = load_w(wv, "wv")
            ck_sb = big.tile([128, L], BF16, tag="ckt")
            nc.sync.dma_start(ck_sb[:], ck[:])
            sk_sb = big.tile([128, L], BF16, tag="skt")
            nc.sync.dma_start(sk_sb[:], sk_t[:])
            # phase-B inputs queue behind phase A's (prefetch during A)
            wq_sb = load_w(wq, "wq")
            wqr_sb = load_w(wqr, "wqr")
            cq_sb = big.tile([128, L], BF16, tag="cqt")
            nc.sync.dma_start(cq_sb[:], cq[:])
            sq_sb = big.tile([128, L], BF16, tag="sqt")
            nc.sync.dma_start(sq_sb[:], sq_t[:])
            xq_sb = []
            for dc in range(8):
                x = big.tile([128, L], BF16, tag=f"xq{dc}", name=f"xq{dc}")
                nc.sync.dma_start(x[:], xqT[dc * 128:(dc + 1) * 128, :])
                xq_sb.append(x)

            # ---- Phase A: k/v projections (xkv by seq halves) ----
            with ExitStack() as actx:
                apool = actx.enter_context(tc.tile_pool(name="a_sb", bufs=1))
                aps = actx.enter_context(tc.tile_pool(name="a_ps", bufs=1,
                                                      space="PSUM"))
                eps_k = apool.tile([2, 1], F32, tag="epsk")
                nc.gpsimd.memset(eps_k[:], 64.0 * EPS)
                for lh in range(2):
                    if lh == 0:
                        xkv_h = xkv_h0
                    else:
                        xkv_h = []
                        for dc in range(8):
                            x = xpool.tile([128, 1024], BF16, tag=f"xkv{dc}",
                                           bufs=1)
                            nc.sync.dma_start(
                                x[:], xkvT[dc * 128:(dc + 1) * 128,
                                           1024:2048])
                            xkv_h.append(x)
                    for fb in range(4):
                        for qn in range(2):
                            col0 = lh * 1024 + qn * 512
                            ps, rot = _proj_pair(
                                nc, aps,
                                [wk_sb[dc][:, fb * 128:(fb + 1) * 128]
                                 for dc in range(8)],
                                [wkr_sb[dc][:, fb * 128:(fb + 1) * 128]
                                 for dc in range(8)],
                                [xkv_h[dc][:, qn * 512:(qn + 1) * 512]
                                 for dc in range(8)])
                            _norm_rope_chunk(tc, apool, aps, kT, ps, rot,
                                             ck_sb, sk_sb, bdiag, bmap,
                                             eps_k, "k", fb, col0, rk_dram)
                    for lc in range(8):
                        kc = lh * 8 + lc
                        ps = aps.tile([128, 512], F32, tag="proj", bufs=2)
                        for dc in range(8):
                            nc.tensor.matmul(
                                ps[:],
                                xkv_h[dc][:, lc * 128:(lc + 1) * 128],
                                wv_sb[dc][:],
                                start=(dc == 0), stop=(dc == 7))
                        va = vaug[kc]
                        nc.gpsimd.memset(va[:], 1.0)
                        va3 = va.rearrange("p (h c) -> p h c", c=65)
                        ps3 = ps.rearrange("p (h c) -> p h c", c=64)
                        nc.vector.tensor_copy(va3[:, :, 0:64], ps3[:])

            # ---- Phase B: q projection, fb-outer so qT[0] finishes first ----
            with ExitStack() as bctx:
                bpool = bctx.enter_context(tc.tile_pool(name="b_sb", bufs=1))
                bps_p = bctx.enter_context(tc.tile_pool(name="b_ps", bufs=1,
                                                        space="PSUM"))
                eps_q = bpool.tile([2, 1], F32, tag="epsq")
                nc.gpsimd.memset(eps_q[:], EPS)
                for fb in range(4):
                    for cn in range(4):   # 512-col chunks across full L
                        col0 = cn * 512
                        ps, rot = _proj_pair(
                            nc, bps_p,
                            [wq_sb[dc][:, fb * 128:(fb + 1) * 128]
                             for dc in range(8)],
                            [wqr_sb[dc][:, fb * 128:(fb + 1) * 128]
                             for dc in range(8)],
                            [xq_sb[dc][:, col0:col0 + 512]
                             for dc in range(8)])
                        _norm_rope_chunk(tc, bpool, bps_p, qT, ps, rot,
                                         cq_sb, sq_sb, bdiag, bmap,
                                         eps_q, "q", fb, col0, None)

            # ---- Phases C+D persistents ----
            p2 = ctx.enter_context(tc.tile_pool(name="p2", bufs=1))
            ytr = [p2.tile([128, L], BF16, tag=f"ytr{i}", name=f"ytr{i}")
                   for i in range(4)]
            sums_g = [p2.tile([128, L], F32, tag=f"sums{g}", name=f"sums{g}")
                      for g in range(2)]
            nc.gpsimd.memset(sums_g[0][:], 1.0)
            nc.gpsimd.memset(sums_g[1][:], 1.0)
            rs_g = [p2.tile([128, L], BF16, tag=f"rs{g}", name=f"rs{g}")
                    for g in range(2)]

            # ---- Phase C: attention (2 q-passes per head) ----
            with ExitStack() as cctx:
                cpool = cctx.enter_context(tc.tile_pool(name="att_sb", bufs=1))
                cps = cctx.enter_context(
                    tc.tile_pool(name="att_ps", bufs=1, space="PSUM"))
                rk_sb = cpool.tile([128, HC, 16], F32, tag="rk")
                nc.sync.dma_start(
                    rk_sb[:], rk_dram.rearrange("h (kc p) -> p h kc", p=128))
                for h in range(HC):
                    fb, off = h // 2, (h % 2) * 64
                    for pss in range(2):
                        yps = [cps.tile([65, 512], F32, tag=f"y{j}", bufs=1,
                                        name=f"yps{h}_{pss}_{j}")
                               for j in range(2)]
                        for kc in range(16):
                            va3 = vaug[kc].rearrange("p (h c) -> p h c", c=65)
                            rk_ap = rk_sb[:, h, kc:kc + 1]
                            sps = cps.tile([128, 1024], F32, tag="s", bufs=3)
                            for j in range(2):
                                qn = pss * 2 + j
                                nc.tensor.matmul(
                                    sps[:, j * 512:(j + 1) * 512],
                                    kT[fb][off:off + 64,
                                           kc * 128:(kc + 1) * 128],
                                    qT[fb][off:off + 64,
                                           qn * 512:(qn + 1) * 512],
                                    start=True, stop=True)
                            pt = cpool.tile([128, 1024], BF16, tag="p", bufs=4)
                            nc.scalar.activation(pt[:], sps[:], EXP,
                                                 scale=rk_ap)
                            for j in range(2):
                                nc.tensor.matmul(
                                    yps[j][:], va3[:, h, :],
                                    pt[:, j * 512:(j + 1) * 512],
                                    start=(kc == 0), stop=(kc == 15))
                        for j in range(2):
                            qn = pss * 2 + j
                            nc.vector.tensor_copy(
                                ytr[fb][off:off + 64,
                                        qn * 512:(qn + 1) * 512],
                                yps[j][0:64, :])
                            slot = 32 * (h % 4)
                            nc.vector.tensor_copy(
                                sums_g[h // 4][slot:slot + 1,
                                               qn * 512:(qn + 1) * 512],
                                yps[j][64:65, :])
                    if h == 3 or h == 7:
                        g = h // 4
                        for hf in range(2):
                            cols = slice(hf * 1024, (hf + 1) * 1024)
                            rs32 = cpool.tile([128, 1024], F32, tag="rs32",
                                              bufs=1, name=f"rs32_{g}_{hf}")
                            nc.vector.reciprocal_approx_fast(
                                out=rs32[:], in_=sums_g[g][:, cols])
                            nc.vector.tensor_copy(rs_g[g][:, cols], rs32[:])

            # ---- Phase D: normalize + output projection ----
            with ExitStack() as dctx:
                dpool = dctx.enter_context(tc.tile_pool(name="out_sb", bufs=1))
                dps = dctx.enter_context(
                    tc.tile_pool(name="out_ps", bufs=1, space="PSUM"))
                wo_sb = []
                for fc in range(4):
                    w = dpool.tile([128, D], BF16, tag=f"wo{fc}")
                    nc.sync.dma_start(w[:], wout[fc * 128:(fc + 1) * 128, :])
                    wo_sb.append(w)
                sel_sb = []
                for i, sd in enumerate((selA_d, selB_d)):
                    s = dpool.tile([128, 128], BF16, tag=f"sel{i}",
                                   name=f"sel{i}")
                    nc.sync.dma_start(s[:], sd[:])
                    sel_sb.append(s)
                for qn in range(4):
                    for fb in range(4):
                        bps = dps.tile([128, 512], F32, tag="bc2", bufs=2)
                        nc.tensor.matmul(
                            bps[:],
                            sel_sb[fb % 2][:],
                            rs_g[fb // 2][:, qn * 512:(qn + 1) * 512],
                            start=True, stop=True)
                        nc.vector.tensor_mul(
                            ytr[fb][:, qn * 512:(qn + 1) * 512],
                            ytr[fb][:, qn * 512:(qn + 1) * 512], bps[:])
                    for nb in range(8):
                        ps = dps.tile([128, 512], F32, tag="oproj", bufs=3)
                        for fc in range(4):
                            nc.tensor.matmul(
                                ps[:],
                                wo_sb[fc][:, nb * 128:(nb + 1) * 128],
                                ytr[fc][:, qn * 512:(qn + 1) * 512],
                                start=(fc == 0), stop=(fc == 3))
                        ot = dpool.tile([128, 512], BF16, tag="ot", bufs=3)
                        nc.scalar.activation(ot[:], ps[:], COPY)
                        nc.sync.dma_start(
                            outT[nb * 128:(nb + 1) * 128,
                                 qn * 512:(qn + 1) * 512], ot[:])
    nc.compile()
    return nc


def get_nc():
    global _NC
    if _NC is None:
        _NC = _build_program()
    return _NC


# --------------------------------------------------------------------------- #
# Host side
# --------------------------------------------------------------------------- #

def _rope_tables(pos, g):
    """Feature-major folded RoPE(+gain) tables, replicated for a 2-head tile."""
    pos = np.asarray(pos).astype(np.float32)
    g = np.asarray(g, dtype=np.float32)
    inv = (1.0 / (10000.0 ** (np.arange(0, DH, 2, dtype=np.float32)
                              / np.float32(DH)))).astype(np.float32)
    ang = pos[:, None] * inv[None, :]                      # (L, 32)
    cos, sin = np.cos(ang, dtype=np.float32), np.sin(ang, dtype=np.float32)
    j = np.arange(DH)
    C = (g[j][:, None] * cos[:, j % 32].T).astype(np.float32)       # (64, L)
    sign = np.where(j < 32, -1.0, 1.0).astype(np.float32)
    S = (sign[:, None] * g[(j + 32) % 64][:, None]
         * sin[:, j % 32].T).astype(np.float32)
    BF = ml_dtypes.bfloat16
    return (np.ascontiguousarray(np.tile(C, (2, 1))).astype(BF),
            np.ascontiguousarray(np.tile(S, (2, 1))).astype(BF))  # (128, L)


def _rot_cols(w):
    """Swap the two 32-col halves of each 64-col head block: W @ P^T."""
    w3 = w.reshape(D, HC, 2, 32)
    return np.ascontiguousarray(w3[:, :, ::-1, :].reshape(D, F))


def make_in_maps(queries, kv, Wq, Wkv, Wout, g_q, g_k, pos_q, pos_k):
    BF = ml_dtypes.bfloat16
    queries = np.asarray(queries, dtype=np.float32)
    kv = np.asarray(kv, dtype=np.float32)
    Wq = np.asarray(Wq, dtype=np.float32)
    Wkv = np.asarray(Wkv, dtype=np.float32)
    Wout = np.asarray(Wout, dtype=np.float32)

    cq, sq = _rope_tables(pos_q, g_q)
    ck, sk = _rope_tables(pos_k, g_k)
    bdiag = np.zeros((128, 2), BF)
    bdiag[0:64, 0] = 1.0
    bdiag[64:128, 1] = 1.0
    bmap = np.zeros((2, 128), BF)
    bmap[0, 0:64] = 1.0
    bmap[1, 64:128] = 1.0
    selA = np.zeros((128, 128), BF)
    selA[0, 0:64] = 1.0
    selA[32, 64:128] = 1.0
    selB = np.zeros((128, 128), BF)
    selB[64, 0:64] = 1.0
    selB[96, 64:128] = 1.0

    Wkv3 = Wkv.reshape(D, 16, 2 * DH)
    xqT_b = [np.ascontiguousarray(queries[b].T.astype(BF)) for b in range(4)]
    xkvT_b = [np.ascontiguousarray(kv[b].T.astype(BF)) for b in range(4)]
    in_maps = []
    for c in range(N_CORES):
        b, grp = c // 2, c % 2
        hs = slice(grp * HC, (grp + 1) * HC)
        wq_g = Wq[:, grp * F:(grp + 1) * F]
        wk_g = Wkv3[:, hs, :DH].reshape(D, F)
        in_maps.append({
            "xqT": xqT_b[b],
            "xkvT": xkvT_b[b],
            "wq": np.ascontiguousarray(wq_g.astype(BF)),
            "wqr": _rot_cols(wq_g).astype(BF),
            "wk": np.ascontiguousarray(wk_g.astype(BF)),
            "wkr": _rot_cols(wk_g).astype(BF),
            "wv": np.ascontiguousarray(
                Wkv3[:, hs, DH:].reshape(D, F).astype(BF)),
            "wout": np.ascontiguousarray(
                Wout[grp * F:(grp + 1) * F, :].astype(BF)),
            "cq": cq, "sq": sq, "ck": ck, "sk": sk,
            "bdiag": bdiag, "bmap": bmap, "selA": selA, "selB": selB,
        })
    return in_maps


def kernel(queries, kv, Wq, Wkv, Wout, g_q, g_k, pos_q, pos_k):
    global LAST_RESULTS
    nc = get_nc()
    in_maps = make_in_maps(queries, kv, Wq, Wkv, Wout, g_q, g_k, pos_q, pos_k)
    trace = bool(int(os.environ.get("KERNEL_TRACE", "0")))
    kw = {}
    if trace:
        kw["tmpdir"] = os.environ.get("KERNEL_TRACE_DIR") or None
    res = run_bass_kernel_spmd(nc, in_maps, core_ids=list(range(N_CORES)),
                               trace=trace, **kw)
    LAST_RESULTS = res
    out = np.empty((4, L, D), np.float32)
    for b in range(4):
        out[b] = (res.results[2 * b]["outT"].astype(np.float32)
                  + res.results[2 * b + 1]["outT"].astype(np.float32)).T
    return out
